# revision 9
# baseline (speedup 1.0000x reference)
"""Bahdanau-attention scoring kernel for Trainium2 (8 NeuronCores, SPMD).

Computes softmax_s( v . tanh(hidden @ Wh^T + enc @ We^T + b) ) for
hidden [32,1024], enc [32,2048,1024]  ->  out [32,2048].

Sharding: data-parallel over batch (4 rows / core). Weights replicated.
Per core: energy computed in [o_part, s_free] layout via fp32r matmuls
(w_eT stationary tiles, enc^T moving tiles, pre-transposed on host),
tanh fused with the per-(o,b) bias q = hidden@Wh^T + b on ScalarE,
v-dot as K=o matmuls into a [1, s] PSUM row, softmax along free dim.
"""

import os
from contextlib import ExitStack

import numpy as np

import concourse.bacc as bacc
import concourse.bass as bass
import concourse.mybir as mybir
import concourse.tile as tile
from concourse.bass_utils import run_bass_kernel_spmd

HID = 1024
BATCH = 32
SRC = 2048
NCORES = 8
BLOC = BATCH // NCORES  # 4 batch rows per core
KT = HID // 128  # 8 k-tiles over the hidden/contraction dim
MT = HID // 128  # 8 m-tiles over the output-feature dim
NCHUNK = 512  # matmul moving free dim / psum bank width (fp32)
SCHUNKS = SRC // NCHUNK  # 4 s-chunks per batch row

F32 = mybir.dt.float32
F32R = mybir.dt.float32r

_compiled = {}
_last_results = None


def _build_kernel(ctx: ExitStack, tc: tile.TileContext, aps: dict):
    nc = tc.nc
    enc_d = aps["enc_t"]  # [BLOC, KT, 128, SRC] (b, k, p, s)
    we_d = aps["w_et"]  # [128, KT, HID]  (p, k, o)
    wh_d = aps["w_ht"]  # [128, KT, HID]
    hid_d = aps["hid_t"]  # [128, KT, BLOC]
    v_d = aps["v_t"]  # [128, MT]
    b_d = aps["b_t"]  # [1, HID]
    out_d = aps["out"]  # [1, BLOC * SRC]

    w_pool = ctx.enter_context(tc.tile_pool(name="w", bufs=1))
    small_pool = ctx.enter_context(tc.tile_pool(name="small", bufs=1))
    enc_pool = ctx.enter_context(tc.tile_pool(name="enc", bufs=2))
    tanh_pool = ctx.enter_context(tc.tile_pool(name="tanh", bufs=12))
    score_pool = ctx.enter_context(tc.tile_pool(name="score", bufs=2))
    prob_pool = ctx.enter_context(tc.tile_pool(name="prob", bufs=2))
    stat_pool = ctx.enter_context(tc.tile_pool(name="stat", bufs=8))
    psum_e = ctx.enter_context(tc.tile_pool(name="psum_e", bufs=6, space="PSUM"))
    psum_s = ctx.enter_context(tc.tile_pool(name="psum_s", bufs=2, space="PSUM"))

    # --- resident weights / small tensors -------------------------------
    w_sb = w_pool.tile([128, KT, HID], F32R)
    nc.sync.dma_start(out=w_sb[:], in_=we_d[:])
    wh_sb = w_pool.tile([128, KT, HID], F32R)
    nc.sync.dma_start(out=wh_sb[:], in_=wh_d[:])
    hid_sb = small_pool.tile([128, KT, BLOC], F32R)
    nc.sync.dma_start(out=hid_sb[:], in_=hid_d[:])
    v_sb = small_pool.tile([128, MT], F32R)
    nc.sync.dma_start(out=v_sb[:], in_=v_d[:])
    b_sb = small_pool.tile([1, HID], F32R)
    nc.sync.dma_start(out=b_sb[:], in_=b_d[:])
    ones_sb = small_pool.tile([1, BLOC], F32R)
    nc.sync.dma_start(out=ones_sb[:], in_=aps["ones_t"][:])

    # --- q[o, b] = Wh @ hidden^T + attn_b (per-partition bias for tanh) --
    q_sb = small_pool.tile([128, MT * BLOC], F32)  # col = m*BLOC + b
    for m in range(MT):
        qp = psum_e.tile([128, NCHUNK], F32, tag="ep")
        for k in range(KT):
            nc.tensor.matmul(
                qp[:, 0:BLOC],
                lhsT=wh_sb[:, k, m * 128 : (m + 1) * 128],
                rhs=hid_sb[:, k, :],
                start=(k == 0),
                stop=False,
            )
        # += attn_b[o] * ones[b]  (K=1 outer product adds the bias)
        nc.tensor.matmul(
            qp[:, 0:BLOC],
            lhsT=b_sb[0:1, m * 128 : (m + 1) * 128],
            rhs=ones_sb[0:1, :],
            start=False,
            stop=True,
        )
        nc.scalar.copy(q_sb[:, m * BLOC : (m + 1) * BLOC], qp[:, 0:BLOC])

    # --- main loop: 16 chunks of 512 s-values ---------------------------
    for b in range(BLOC):
        score_sb = score_pool.tile([1, SRC], F32)
        for s in range(SCHUNKS):
            s0 = s * NCHUNK
            enc_sb = enc_pool.tile([128, KT, NCHUNK], F32R)
            nc.sync.dma_start(
                out=enc_sb[:],
                in_=enc_d[b].rearrange("k p s -> p k s")[:, :, s0 : s0 + NCHUNK],
            )
            th_tiles = []
            for m in range(MT):
                ep = psum_e.tile([128, NCHUNK], F32, tag="ep", name="ep")
                for k in range(KT):
                    nc.tensor.matmul(
                        ep[:],
                        lhsT=w_sb[:, k, m * 128 : (m + 1) * 128],
                        rhs=enc_sb[:, k, :],
                        start=(k == 0),
                        stop=(k == KT - 1),
                    )
                th = tanh_pool.tile([128, NCHUNK], F32R)
                nc.scalar.activation(
                    th[:],
                    ep[:],
                    mybir.ActivationFunctionType.Tanh,
                    bias=q_sb[:, m * BLOC + b : m * BLOC + b + 1],
                    scale=1.0,
                )
                th_tiles.append(th)
            sp = psum_s.tile([1, NCHUNK], F32)
            for m in range(MT):
                nc.tensor.matmul(
                    sp[:],
                    lhsT=v_sb[:, m : m + 1],
                    rhs=th_tiles[m][:],
                    start=(m == 0),
                    stop=(m == MT - 1),
                )
            nc.vector.tensor_copy(score_sb[0:1, s0 : s0 + NCHUNK], sp[:])

        # --- softmax over this batch row's 2048 scores ------------------
        mx = stat_pool.tile([1, 1], F32)
        nc.vector.tensor_reduce(
            mx[:], score_sb[:], axis=mybir.AxisListType.X, op=mybir.AluOpType.max
        )
        negmx = stat_pool.tile([1, 1], F32)
        nc.vector.tensor_scalar_mul(negmx[:], mx[:], -1.0)
        prob_sb = prob_pool.tile([1, SRC], F32)
        esum = stat_pool.tile([1, 1], F32)
        nc.scalar.activation(
            prob_sb[:],
            score_sb[:],
            mybir.ActivationFunctionType.Exp,
            bias=negmx[0:1, 0:1],
            scale=1.0,
            accum_out=esum[:],
        )
        rcp = stat_pool.tile([1, 1], F32)
        nc.vector.reciprocal(rcp[:], esum[:])
        nc.vector.tensor_scalar_mul(prob_sb[:], prob_sb[:], rcp[0:1, 0:1])
        nc.sync.dma_start(out=out_d[0:1, b * SRC : (b + 1) * SRC], in_=prob_sb[:])


def build_nc():
    nc = bacc.Bacc("TRN2", target_bir_lowering=False, debug=False)
    aps = {
        "enc_t": nc.dram_tensor(
            "enc_t", [BLOC, KT, 128, SRC], F32R, kind="ExternalInput"
        ).ap(),
        "w_et": nc.dram_tensor(
            "w_et", [128, KT, HID], F32R, kind="ExternalInput"
        ).ap(),
        "w_ht": nc.dram_tensor(
            "w_ht", [128, KT, HID], F32R, kind="ExternalInput"
        ).ap(),
        "hid_t": nc.dram_tensor(
            "hid_t", [128, KT, BLOC], F32R, kind="ExternalInput"
        ).ap(),
        "v_t": nc.dram_tensor("v_t", [128, MT], F32R, kind="ExternalInput").ap(),
        "b_t": nc.dram_tensor("b_t", [1, HID], F32R, kind="ExternalInput").ap(),
        "ones_t": nc.dram_tensor(
            "ones_t", [1, BLOC], F32R, kind="ExternalInput"
        ).ap(),
        "out": nc.dram_tensor(
            "out", [1, BLOC * SRC], F32, kind="ExternalOutput"
        ).ap(),
    }
    with tile.TileContext(nc) as tc, ExitStack() as ctx:
        _build_kernel(ctx, tc, aps)
    nc.compile()
    return nc


def _prep_shared(hidden, attn_w, attn_b, v):
    w_e_t = np.ascontiguousarray(attn_w[:, HID:].T)  # [h, o]
    w_h_t = np.ascontiguousarray(attn_w[:, :HID].T)  # [h, o]
    # [h, o] -> [kt, 128, o] -> [128, kt, o]
    w_et = np.ascontiguousarray(w_e_t.reshape(KT, 128, HID).transpose(1, 0, 2))
    w_ht = np.ascontiguousarray(w_h_t.reshape(KT, 128, HID).transpose(1, 0, 2))
    v_t = np.ascontiguousarray(v.reshape(MT, 128).T)  # [128, mt]
    b_t = np.ascontiguousarray(attn_b.reshape(1, HID))
    hid_all = []
    for c in range(NCORES):
        ht = hidden[c * BLOC : (c + 1) * BLOC].T  # [h, bloc]
        hid_all.append(
            np.ascontiguousarray(ht.reshape(KT, 128, BLOC).transpose(1, 0, 2))
        )
    return w_et, w_ht, v_t, b_t, hid_all


def kernel(hidden, encoder_outputs, attn_w, attn_b, v):
    global _last_results
    hidden = np.asarray(hidden, dtype=np.float32)
    encoder_outputs = np.asarray(encoder_outputs, dtype=np.float32)
    attn_w = np.asarray(attn_w, dtype=np.float32)
    attn_b = np.asarray(attn_b, dtype=np.float32)
    v = np.asarray(v, dtype=np.float32)

    if "nc" not in _compiled:
        _compiled["nc"] = build_nc()
    nc = _compiled["nc"]

    w_et, w_ht, v_t, b_t, hid_all = _prep_shared(hidden, attn_w, attn_b, v)
    in_maps = []
    for c in range(NCORES):
        enc_c = encoder_outputs[c * BLOC : (c + 1) * BLOC]  # [bloc, s, h]
        # [bloc, s, h] -> [bloc, h, s] -> [bloc, kt, 128, s]
        enc_t = np.ascontiguousarray(enc_c.transpose(0, 2, 1)).reshape(
            BLOC, KT, 128, SRC
        )
        in_maps.append(
            {
                "enc_t": enc_t,
                "w_et": w_et,
                "w_ht": w_ht,
                "hid_t": hid_all[c],
                "v_t": v_t,
                "b_t": b_t,
                "ones_t": np.ones((1, BLOC), dtype=np.float32),
            }
        )

    res = run_bass_kernel_spmd(nc, in_maps, list(range(NCORES)))
    _last_results = res
    out = np.concatenate(
        [res.results[c]["out"].reshape(BLOC, SRC) for c in range(NCORES)], axis=0
    )
    return out.astype(np.float32)


# revision 10
# speedup vs baseline: 1.1338x; 1.1338x over previous
"""Bahdanau-attention scoring kernel for Trainium2 (8 NeuronCores, SPMD).

Computes softmax_s( v . tanh(hidden @ Wh^T + enc @ We^T + b) ) for
hidden [32,1024], enc [32,2048,1024]  ->  out [32,2048].

Sharding: data-parallel over batch (4 rows / core). Weights replicated.
Per core: energy computed in [o_part, s_free] layout via fp16 matmuls
(w_eT stationary tiles, enc^T moving tiles, pre-transposed + cast on
host), tanh fused with the per-(o,b) bias q = hidden@Wh^T + b on
ScalarE, v-dot as K=o matmuls into a [1, s] PSUM row, softmax along the
free dim. fp32 PSUM accumulation throughout; fp16 operand rounding only
(~1.5e-3 rel err vs fp32 reference).
"""

from contextlib import ExitStack

import numpy as np

import concourse.bacc as bacc
import concourse.mybir as mybir
import concourse.tile as tile
from concourse.bass_utils import run_bass_kernel_spmd

HID = 1024
BATCH = 32
SRC = 2048
NCORES = 8
BLOC = BATCH // NCORES  # 4 batch rows per core
KT = HID // 128  # 8 k-tiles over the contraction dim
MT = HID // 128  # 8 m-tiles over the output-feature dim
NCHUNK = 512  # matmul moving free dim / psum bank width (fp32 out)
SCHUNKS = SRC // NCHUNK  # 4 s-chunks per batch row

F32 = mybir.dt.float32
F16 = mybir.dt.float16

_compiled = {}
_last_results = None


def _build_kernel(ctx: ExitStack, tc: tile.TileContext, aps: dict):
    nc = tc.nc
    enc_d = aps["enc_t"]  # [BLOC, KT, 128, SRC] (b, k, p, s) fp16
    we_d = aps["w_et"]  # [128, KT, HID]  (p, k, o) fp16
    wh_d = aps["w_ht"]  # [128, KT, HID] fp16
    hid_d = aps["hid_t"]  # [128, KT, BLOC] fp16
    v_d = aps["v_t"]  # [128, MT] fp16
    b_d = aps["b_t"]  # [1, HID] fp16
    ones_d = aps["ones_t"]  # [1, BLOC] fp16
    out_d = aps["out"]  # [1, BLOC * SRC] fp32

    w_pool = ctx.enter_context(tc.tile_pool(name="w", bufs=1))
    small_pool = ctx.enter_context(tc.tile_pool(name="small", bufs=1))
    enc_pool = ctx.enter_context(tc.tile_pool(name="enc", bufs=3))
    tanh_pool = ctx.enter_context(tc.tile_pool(name="tanh", bufs=12))
    score_pool = ctx.enter_context(tc.tile_pool(name="score", bufs=2))
    prob_pool = ctx.enter_context(tc.tile_pool(name="prob", bufs=2))
    stat_pool = ctx.enter_context(tc.tile_pool(name="stat", bufs=4))
    psum_e = ctx.enter_context(tc.tile_pool(name="psum_e", bufs=6, space="PSUM"))
    psum_s = ctx.enter_context(tc.tile_pool(name="psum_s", bufs=2, space="PSUM"))

    # --- tiny resident tensors first (cheap DMAs) -----------------------
    hid_sb = small_pool.tile([128, KT, BLOC], F16)
    nc.sync.dma_start(out=hid_sb[:], in_=hid_d[:])
    v_sb = small_pool.tile([128, MT], F16)
    nc.sync.dma_start(out=v_sb[:], in_=v_d[:])
    b_sb = small_pool.tile([1, HID], F16)
    nc.sync.dma_start(out=b_sb[:], in_=b_d[:])
    ones_sb = small_pool.tile([1, BLOC], F16)
    nc.sync.dma_start(out=ones_sb[:], in_=ones_d[:])

    # wh per-k so the q matmuls can start as soon as k-slices land
    wh_sb = w_pool.tile([128, KT, HID], F16)
    for k in range(KT):
        nc.sync.dma_start(out=wh_sb[:, k, :], in_=wh_d[:, k, :])

    # --- q[o, b] = Wh @ hidden^T + attn_b (per-partition bias for tanh) --
    q_sb = small_pool.tile([128, MT * BLOC], F32)  # col = m*BLOC + b
    for m in range(MT):
        qp = psum_e.tile([128, NCHUNK], F32, tag="ep", name="qp")
        for k in range(KT):
            nc.tensor.matmul(
                qp[:, 0:BLOC],
                lhsT=wh_sb[:, k, m * 128 : (m + 1) * 128],
                rhs=hid_sb[:, k, :],
                start=(k == 0),
                stop=False,
            )
        # += attn_b[o] * ones[b]  (K=1 outer product adds the bias)
        nc.tensor.matmul(
            qp[:, 0:BLOC],
            lhsT=b_sb[0:1, m * 128 : (m + 1) * 128],
            rhs=ones_sb[0:1, :],
            start=False,
            stop=True,
        )
        nc.scalar.copy(q_sb[:, m * BLOC : (m + 1) * BLOC], qp[:, 0:BLOC])

    # --- w_e and the first enc chunk, interleaved per-k -----------------
    w_sb = w_pool.tile([128, KT, HID], F16)
    enc0_sb = enc_pool.tile([128, KT, NCHUNK], F16, tag="enc", name="enc0_sb")
    for k in range(KT):
        nc.sync.dma_start(out=w_sb[:, k, :], in_=we_d[:, k, :])
        nc.sync.dma_start(
            out=enc0_sb[:, k, :],
            in_=enc_d[0].rearrange("k p s -> p k s")[:, k, 0:NCHUNK],
        )

    # --- main loop: 16 chunks of 512 s-values ---------------------------
    for b in range(BLOC):
        score_sb = score_pool.tile([1, SRC], F32)
        pmax = stat_pool.tile([1, SCHUNKS], F32, name="pmax")
        for s in range(SCHUNKS):
            s0 = s * NCHUNK
            if b == 0 and s == 0:
                enc_sb = enc0_sb
            else:
                enc_sb = enc_pool.tile([128, KT, NCHUNK], F16, tag="enc")
                nc.sync.dma_start(
                    out=enc_sb[:],
                    in_=enc_d[b].rearrange("k p s -> p k s")[:, :, s0 : s0 + NCHUNK],
                )
            th_tiles = []
            for m in range(MT):
                ep = psum_e.tile([128, NCHUNK], F32, tag="ep", name="ep")
                for k in range(KT):
                    nc.tensor.matmul(
                        ep[:],
                        lhsT=w_sb[:, k, m * 128 : (m + 1) * 128],
                        rhs=enc_sb[:, k, :],
                        start=(k == 0),
                        stop=(k == KT - 1),
                    )
                th = tanh_pool.tile([128, NCHUNK], F16)
                nc.scalar.activation(
                    th[:],
                    ep[:],
                    mybir.ActivationFunctionType.Tanh,
                    bias=q_sb[:, m * BLOC + b : m * BLOC + b + 1],
                    scale=1.0,
                )
                th_tiles.append(th)
            sp = psum_s.tile([1, NCHUNK], F32)
            for m in range(MT):
                nc.tensor.matmul(
                    sp[:],
                    lhsT=v_sb[:, m : m + 1],
                    rhs=th_tiles[m][:],
                    start=(m == 0),
                    stop=(m == MT - 1),
                )
            nc.vector.tensor_copy(score_sb[0:1, s0 : s0 + NCHUNK], sp[:])
            nc.vector.tensor_reduce(
                pmax[0:1, s : s + 1],
                score_sb[0:1, s0 : s0 + NCHUNK],
                axis=mybir.AxisListType.X,
                op=mybir.AluOpType.max,
            )

        # --- softmax over this batch row's 2048 scores ------------------
        negmx = stat_pool.tile([1, 1], F32)
        nc.vector.tensor_reduce(
            negmx[:], pmax[:], axis=mybir.AxisListType.X, op=mybir.AluOpType.max
        )
        nc.vector.tensor_scalar_mul(negmx[:], negmx[:], -1.0)
        prob_sb = prob_pool.tile([1, SRC], F32)
        esum = stat_pool.tile([1, 1], F32)
        nc.scalar.activation(
            prob_sb[:],
            score_sb[:],
            mybir.ActivationFunctionType.Exp,
            bias=negmx[0:1, 0:1],
            scale=1.0,
            accum_out=esum[:],
        )
        rcp = stat_pool.tile([1, 1], F32)
        nc.vector.reciprocal(rcp[:], esum[:])
        nc.vector.tensor_scalar_mul(prob_sb[:], prob_sb[:], rcp[0:1, 0:1])
        nc.sync.dma_start(out=out_d[0:1, b * SRC : (b + 1) * SRC], in_=prob_sb[:])


def build_nc():
    nc = bacc.Bacc("TRN2", target_bir_lowering=False, debug=False)
    aps = {
        "enc_t": nc.dram_tensor(
            "enc_t", [BLOC, KT, 128, SRC], F16, kind="ExternalInput"
        ).ap(),
        "w_et": nc.dram_tensor(
            "w_et", [128, KT, HID], F16, kind="ExternalInput"
        ).ap(),
        "w_ht": nc.dram_tensor(
            "w_ht", [128, KT, HID], F16, kind="ExternalInput"
        ).ap(),
        "hid_t": nc.dram_tensor(
            "hid_t", [128, KT, BLOC], F16, kind="ExternalInput"
        ).ap(),
        "v_t": nc.dram_tensor("v_t", [128, MT], F16, kind="ExternalInput").ap(),
        "b_t": nc.dram_tensor("b_t", [1, HID], F16, kind="ExternalInput").ap(),
        "ones_t": nc.dram_tensor(
            "ones_t", [1, BLOC], F16, kind="ExternalInput"
        ).ap(),
        "out": nc.dram_tensor(
            "out", [1, BLOC * SRC], F32, kind="ExternalOutput"
        ).ap(),
    }
    with tile.TileContext(nc) as tc, ExitStack() as ctx:
        _build_kernel(ctx, tc, aps)
    nc.compile()
    return nc


def _prep_shared(hidden, attn_w, attn_b, v):
    w_e_t = np.ascontiguousarray(attn_w[:, HID:].T)  # [h, o]
    w_h_t = np.ascontiguousarray(attn_w[:, :HID].T)  # [h, o]
    # [h, o] -> [kt, 128, o] -> [128, kt, o]
    w_et = np.ascontiguousarray(
        w_e_t.reshape(KT, 128, HID).transpose(1, 0, 2).astype(np.float16)
    )
    w_ht = np.ascontiguousarray(
        w_h_t.reshape(KT, 128, HID).transpose(1, 0, 2).astype(np.float16)
    )
    v_t = np.ascontiguousarray(v.reshape(MT, 128).T.astype(np.float16))  # [128, mt]
    b_t = np.ascontiguousarray(attn_b.reshape(1, HID).astype(np.float16))
    hid_all = []
    for c in range(NCORES):
        ht = hidden[c * BLOC : (c + 1) * BLOC].T  # [h, bloc]
        hid_all.append(
            np.ascontiguousarray(
                ht.reshape(KT, 128, BLOC).transpose(1, 0, 2).astype(np.float16)
            )
        )
    return w_et, w_ht, v_t, b_t, hid_all


def kernel(hidden, encoder_outputs, attn_w, attn_b, v):
    global _last_results
    hidden = np.asarray(hidden, dtype=np.float32)
    encoder_outputs = np.asarray(encoder_outputs, dtype=np.float32)
    attn_w = np.asarray(attn_w, dtype=np.float32)
    attn_b = np.asarray(attn_b, dtype=np.float32)
    v = np.asarray(v, dtype=np.float32)

    if "nc" not in _compiled:
        _compiled["nc"] = build_nc()
    nc = _compiled["nc"]

    w_et, w_ht, v_t, b_t, hid_all = _prep_shared(hidden, attn_w, attn_b, v)
    in_maps = []
    for c in range(NCORES):
        enc_c = encoder_outputs[c * BLOC : (c + 1) * BLOC]  # [bloc, s, h]
        # [bloc, s, h] -> [bloc, h, s] fp16 -> [bloc, kt, 128, s]
        enc_t = (
            np.ascontiguousarray(enc_c.transpose(0, 2, 1))
            .astype(np.float16)
            .reshape(BLOC, KT, 128, SRC)
        )
        in_maps.append(
            {
                "enc_t": enc_t,
                "w_et": w_et,
                "w_ht": w_ht,
                "hid_t": hid_all[c],
                "v_t": v_t,
                "b_t": b_t,
                "ones_t": np.ones((1, BLOC), dtype=np.float16),
            }
        )

    res = run_bass_kernel_spmd(nc, in_maps, list(range(NCORES)))
    _last_results = res
    out = np.concatenate(
        [res.results[c]["out"].reshape(BLOC, SRC) for c in range(NCORES)], axis=0
    )
    return out.astype(np.float32)
